# revision 10
# baseline (speedup 1.0000x reference)
"""Trainium2 Bass kernel for nn_EnhancedGCN42 (4-layer GCN + MLP classifier).

Strategy (8 NeuronCores, SPMD single NEFF):
  - Nodes dst-sharded: device d owns dst nodes [d*12500, (d+1)*12500).
  - A-hat = D^-1/2 (A+I) D^-1/2 factorized: tables store dis*h rows (bf16,
    256B rows); aggregation output scaled by dis_dst.
  - Per layer: per-edge rows gathered via dma_gather (4 SWDGE queues);
    aggregation ps[feat, dst] += g_chunk^T @ S_use where S_use are
    host-precomputed one-hot scatter matrices streamed from DRAM in fp8
    (graph-static, shared by all 4 layers; no on-chip S build).
  - Transposed compute layout throughout: aggregation produces [feat, dst];
    dense W/BN/ReLU run with per-feature affine on partitions; the LAST
    matmul of each phase swaps operand roles to emit [dst, feat] directly
    for the table write (zero PE transposes).
  - Dense weights bf16; BN folded on host; biases injected via K=1 matmuls
    or per-partition activation bias.
  - AllGather halves in separate DRAM tensors so next-phase gathers for
    src-ranges 0-1 depend only on half A; next-phase gathers are issued
    interleaved with the AllGather triggers to eliminate boundary stalls.

kernel(**inputs) -> [100000, 2] float32.
"""
import hashlib
import numpy as np
import ml_dtypes

import concourse.bacc as bacc
import concourse.bass as bass
import concourse.mybir as mybir
import concourse.tile as tile
from concourse.masks import make_identity
from concourse.bass_utils import run_bass_kernel_spmd

f32 = mybir.dt.float32
bf16 = mybir.dt.bfloat16
fp8 = mybir.dt.float8e4
i16 = mybir.dt.int16
nbf16 = ml_dtypes.bfloat16
nfp8 = ml_dtypes.float8_e4m3

P = 128
NDEV = 8
NR = 4           # src index ranges (int16 limit)
EPS = 1e-5
WTAB = 128       # table row = 128 cols bf16 = 256B

PREF_BLK = 4     # next-phase blocks whose r0/r1 gathers are issued before AG-B


def _prep(x, edge_index, params, N):
    """Host preprocessing: graph partition + S matrices + folded constants."""
    SHARD = N // NDEV
    TSHARD = ((SHARD + P - 1) // P) * P
    NT = TSHARD // P
    TROWS = TSHARD * NDEV
    RNGW = TROWS // NR
    assert RNGW <= 32768 and RNGW % P == 0

    ei = edge_index.astype(np.int64)
    loop = np.arange(N, dtype=np.int64)
    src = np.concatenate([ei[0], loop])
    dst = np.concatenate([ei[1], loop])
    deg = np.bincount(dst, minlength=N).astype(np.float32)
    dis = (1.0 / np.sqrt(deg)).astype(np.float32)

    HSH = TSHARD // 2
    HALF = TROWS // 2

    def padrow2(n):
        sh = n // SHARD
        i = n - sh * SHARD
        return np.where(i < HSH, sh * HSH + i, HALF + sh * HSH + (i - HSH))

    psrc = padrow2(src)

    # exclude appended self-loops (handled densely via identity matmul)
    nE = ei.shape[1]
    src_e, dst_e, psrc_e = src[:nE], dst[:nE], psrc[:nE]

    counts = np.zeros((NDEV, NT, NR), dtype=np.int64)
    dev_edges = []
    for d in range(NDEV):
        m = (dst_e >= d * SHARD) & (dst_e < (d + 1) * SHARD)
        es = psrc_e[m]
        el = dst_e[m] - d * SHARD
        t_id = el >> 7
        r_id = es // RNGW
        order = np.lexsort((es, r_id, t_id))  # (tile, range, src-ascending)
        es, el, t_id, r_id = es[order], el[order], t_id[order], r_id[order]
        # unique-src count per (t, r) group (dedup: S carries multiplicity)
        grp_key = t_id * NR + r_id
        first = np.ones(len(es), dtype=bool)
        first[1:] = (es[1:] != es[:-1]) | (grp_key[1:] != grp_key[:-1])
        np.add.at(counts[d], (t_id[first], r_id[first]), 1)
        dev_edges.append((es, el, t_id, r_id, first))

    grp_rows = ((counts.max(axis=0) + 15) // 16 * 16).astype(np.int64)  # [NT, NR]

    BLK = 8
    n_blk = (NT + BLK - 1) // BLK
    grp_off = np.zeros((NT, NR), dtype=np.int64)
    blk_off = np.zeros((n_blk, NR), dtype=np.int64)
    blk_rows = np.zeros((n_blk, NR), dtype=np.int64)
    acc = 0
    for b in range(n_blk):
        for r in range(NR):
            blk_off[b, r] = acc
            for t in range(b * BLK, min((b + 1) * BLK, NT)):
                grp_off[t, r] = acc
                acc += grp_rows[t, r]
            acc = (acc + P - 1) // P * P
            blk_rows[b, r] = acc - blk_off[b, r]
    TOT = acc

    # chunk-use enumeration: per (t, r) the 128-row chunks its group overlaps.
    # Each use = one S matrix column-block [128 rows, 128 dst one-hot].
    uses = [[[] for _ in range(NR)] for _ in range(NT)]  # (local_chunk, use_id, chunk)
    blk_use = np.zeros((n_blk, 2), dtype=np.int64)  # [use_lo, use_hi) per block
    n_uses = 0
    for b in range(n_blk):
        blk_use[b, 0] = n_uses
        for r in range(NR):
            for t in range(b * BLK, min((b + 1) * BLK, NT)):
                g0, g1 = grp_off[t, r], grp_off[t, r] + grp_rows[t, r]
                c0, c1 = int(g0 // P), int((g1 + P - 1) // P)
                for ci in range(c0, c1):
                    uses[t][r].append((ci - int(blk_off[b, r]) // P, n_uses, ci))
                    n_uses += 1
        blk_use[b, 1] = n_uses

    idx_w = np.zeros((NDEV, P, TOT // 16), dtype=np.int16)
    S_w = np.zeros((NDEV, P, n_uses, P), dtype=np.int8)  # multiplicity, per dev
    for d in range(NDEV):
        es, el, t_id, r_id, first = dev_edges[d]
        IDX = np.zeros(TOT, dtype=np.int16)
        Sd = S_w[d]
        # global row of each edge: group offset + rank of its unique src
        uid = np.cumsum(first) - 1          # unique-src ordinal (global running)
        pos = 0
        upos = 0
        for t in range(NT):
            for r in range(NR):
                gmask = (t_id == t) & (r_id == r)
                idxs = np.nonzero(gmask)[0]
                if len(idxs) == 0:
                    continue
                o = int(grp_off[t, r])
                loc_u = uid[idxs] - uid[idxs[0]]       # 0-based unique rank in group
                cu = int(loc_u[-1]) + 1
                u_src = es[idxs[first[idxs]]]          # unique srcs in order
                IDX[o:o + cu] = (u_src - r * RNGW).astype(np.int16)
                g_row = o + loc_u                      # global gather row per edge
                ci = g_row // P
                part = g_row % P
                # chunk -> use id for this (t, r)
                c0 = int(grp_off[t, r] // P)
                du_of = {cc: du for (_lc, du, cc) in uses[t][r]}
                du_arr = np.array([du_of[int(c)] for c in ci], dtype=np.int64)
                np.add.at(Sd, (part, du_arr, el[idxs] - t * P), 1)
        idx_w[d] = np.tile(IDX.reshape(-1, 16).T, (8, 1))

    dis_pad = np.zeros(TROWS, dtype=np.float32)
    for s in range(NDEV):
        dis_pad[s * TSHARD:s * TSHARD + SHARD] = dis[s * SHARD:(s + 1) * SHARD]
    # NOTE: dis_pad above is in linear shard order [dev, TSHARD]; per-device
    # dst dis vector is dis of its own shard (padded).
    dis_dev = np.zeros((NDEV, TSHARD), dtype=np.float32)
    for d in range(NDEV):
        dis_dev[d, :SHARD] = dis[d * SHARD:(d + 1) * SHARD]
    dis_t = np.stack([dis_dev[d].reshape(NT, P).T for d in range(NDEV)])  # [NDEV, P, NT]
    dis_bc = np.stack([np.tile(dis_dev[d][None, :], (P, 1)) for d in range(NDEV)])
    dis_bc = dis_bc.astype(nbf16)  # [NDEV, P, TSHARD]

    # x-tilde table (bf16, padded cols), half-shard-major row space
    xt = np.zeros((TROWS, WTAB), dtype=nbf16)
    v = (dis[:, None] * x).astype(nbf16)
    rows = padrow2(np.arange(N))
    xt[rows, :x.shape[1]] = v

    def fold(g, be, rm, rv, b):
        k = (1.0 / np.sqrt(rv + EPS)).astype(np.float32)
        s = g * k
        t = (b - rm) * s + be
        return s.astype(np.float32), t.astype(np.float32)

    s1, t1 = fold(params["g1"], params["be1"], params["rm1"], params["rv1"], params["b1"])
    s2, t2 = fold(params["g2"], params["be2"], params["rm2"], params["rv2"], params["b2"])
    s3, t3 = fold(params["g3"], params["be3"], params["rm3"], params["rv3"], params["b3"])
    s4, t4 = fold(params["g4"], params["be4"], params["rm4"], params["rv4"], params["b4"])
    zk = (1.0 / np.sqrt(params["crv1"] + EPS)).astype(np.float32)
    cs1 = params["cg1"] * zk
    ct1 = -params["crm1"] * cs1 + params["cbe1"]
    zk = (1.0 / np.sqrt(params["crv2"] + EPS)).astype(np.float32)
    cs2 = params["cg2"] * zk
    ct2 = -params["crm2"] * cs2 + params["cbe2"]
    cW2p = (cs1[:, None] * params["cW2"]).astype(np.float32)
    cb2p = (ct1 @ params["cW2"] + params["cb2"]).astype(np.float32)
    cW3p = (cs2[:, None] * params["cW3"]).astype(np.float32)
    cb3p = (ct2 @ params["cW3"] + params["cb3"]).astype(np.float32)

    # per-partition activation scale/bias vectors
    vecs = np.zeros((P, 12), dtype=np.float32)
    vecs[:, 0], vecs[:, 1] = s2[:128], t2[:128]
    vecs[:, 2], vecs[:, 3] = s2[128:], t2[128:]
    vecs[:, 4], vecs[:, 5] = s3, t3
    vecs[:64, 6], vecs[:64, 7] = s4, t4
    vecs[:64, 8] = params["cb1"]
    vecs[:32, 9] = cb2p
    vecs[:2, 10] = cb3p

    W1s = (params["W1"] * s1[None, :]).astype(np.float32)  # [42, 128]
    t1row = t1[None, :].astype(np.float32)                 # [1, 128]
    W3pack = np.concatenate([params["W3"][:128], params["W3"][128:]], axis=1)

    return dict(
        N=N, SHARD=SHARD, TSHARD=TSHARD, NT=NT, TROWS=TROWS, RNGW=RNGW,
        HSH=HSH, HALF=HALF,
        TOT=TOT, uses=uses, n_uses=n_uses, blk_use=blk_use,
        BLK=BLK, n_blk=n_blk, blk_off=blk_off, blk_rows=blk_rows,
        idx_w=idx_w, S_w=S_w, dis_t=dis_t, dis_bc=dis_bc, xt=xt, vecs=vecs,
        W1s=W1s, t1row=t1row,
        W2=params["W2"].astype(np.float32),
        W3=W3pack.astype(np.float32),
        W4=params["W4"].astype(np.float32),
        cW1=params["cW1"].astype(np.float32), cW2p=cW2p, cW3p=cW3p,
        d_in=x.shape[1],
    )


def _build(meta):
    """Build the Bass program (same for all cores)."""
    NT, TSHARD, TROWS, RNGW = meta["NT"], meta["TSHARD"], meta["TROWS"], meta["RNGW"]
    TOT = meta["TOT"]
    uses, n_uses, blk_use = meta["uses"], meta["n_uses"], meta["blk_use"]
    BLK, n_blk, blk_off, blk_rows = meta["BLK"], meta["n_blk"], meta["blk_off"], meta["blk_rows"]
    D_IN = meta["d_in"]
    HSH = TSHARD // 2
    HALF = TROWS // 2
    NTH = HSH // P  # tiles per half

    nc = bacc.Bacc(None, target_bir_lowering=False, num_swdge_queues=4)
    t_xt = nc.dram_tensor("xt", [TROWS, WTAB], bf16, kind="ExternalInput")
    t_idx = nc.dram_tensor("idx", [P, TOT // 16], i16, kind="ExternalInput")
    t_S = nc.dram_tensor("S", [P, n_uses, P], fp8, kind="ExternalInput")
    t_xto = nc.dram_tensor("xt_own", [TSHARD, WTAB], bf16, kind="ExternalInput")
    t_dis = nc.dram_tensor("dis", [P, NT], f32, kind="ExternalInput")
    t_disbc = nc.dram_tensor("disbc", [P, TSHARD], bf16, kind="ExternalInput")
    t_vecs = nc.dram_tensor("vecs", [P, 12], f32, kind="ExternalInput")
    t_W1s = nc.dram_tensor("W1s", [D_IN, 128], f32, kind="ExternalInput")
    t_t1r = nc.dram_tensor("t1r", [1, 128], f32, kind="ExternalInput")
    t_W2 = nc.dram_tensor("W2", [128, 256], f32, kind="ExternalInput")
    t_W3 = nc.dram_tensor("W3", [128, 256], f32, kind="ExternalInput")  # packed K-halves
    t_W4 = nc.dram_tensor("W4", [128, 64], f32, kind="ExternalInput")
    t_cW1 = nc.dram_tensor("cW1", [64, 64], f32, kind="ExternalInput")
    t_cW2 = nc.dram_tensor("cW2p", [64, 32], f32, kind="ExternalInput")
    t_cW3 = nc.dram_tensor("cW3p", [32, 2], f32, kind="ExternalInput")
    t_out = nc.dram_tensor("outT", [2, TSHARD], f32, kind="ExternalOutput")

    cc_inA = [nc.dram_tensor(f"cc_inA{i}", [HSH, WTAB], bf16) for i in range(3)]
    cc_inB = [nc.dram_tensor(f"cc_inB{i}", [HSH, WTAB], bf16) for i in range(3)]
    tabsA = [nc.dram_tensor(f"tabA{i}", [HALF, WTAB], bf16, addr_space="Shared")
             for i in range(3)]
    tabsB = [nc.dram_tensor(f"tabB{i}", [HALF, WTAB], bf16, addr_space="Shared")
             for i in range(3)]

    qctr = [0]

    def qrr():
        qctr[0] = (qctr[0] + 1) % 4
        return qctr[0]

    with tile.TileContext(nc) as tc:
        with (
            tc.tile_pool(name="const", bufs=1) as cpool,
            tc.tile_pool(name="gp", bufs=10) as gpool,
            tc.tile_pool(name="sp", bufs=2) as spool,
            tc.tile_pool(name="pagg", bufs=3, space="PSUM") as pagg,
            tc.tile_pool(name="paux", bufs=3, space="PSUM") as paux,
            tc.tile_pool(name="ep", bufs=4) as ep,
        ):
            # ---- constants
            idx_sb = cpool.tile([P, TOT // 16], i16)
            nc.sync.dma_start(out=idx_sb[:], in_=t_idx[:])
            dis_sb = cpool.tile([P, NT], f32)
            nc.sync.dma_start(out=dis_sb[:], in_=t_dis[:])
            disbc_sb = cpool.tile([P, TSHARD], bf16)
            nc.sync.dma_start(out=disbc_sb[:], in_=t_disbc[:])
            vecs_sb = cpool.tile([P, 12], f32)
            nc.sync.dma_start(out=vecs_sb[:], in_=t_vecs[:])

            def bconst(tname, shape, name):
                tl = cpool.tile(shape, bf16, name=name)
                tf = cpool.tile(shape, f32, name=name + "f")
                nc.sync.dma_start(out=tf[:], in_=tname[:])
                nc.vector.tensor_copy(out=tl[:], in_=tf[:])
                return tl

            W1s_sb = bconst(t_W1s, [D_IN, 128], "W1s_sb")
            t1r_sb = bconst(t_t1r, [1, 128], "t1r_sb")
            W2_sb = bconst(t_W2, [128, 256], "W2_sb")
            W3_sb = bconst(t_W3, [128, 256], "W3_sb")
            W4_sb = bconst(t_W4, [128, 64], "W4_sb")
            cW1_sb = bconst(t_cW1, [64, 64], "cW1_sb")
            cW2_sb = bconst(t_cW2, [64, 32], "cW2_sb")
            cW3_sb = bconst(t_cW3, [32, 2], "cW3_sb")
            ident = cpool.tile([P, P], f32)
            make_identity(nc, ident[:])
            ident_bf = cpool.tile([P, P], bf16)
            nc.vector.tensor_copy(out=ident_bf[:], in_=ident[:])
            ones1 = cpool.tile([1, P], bf16)
            nc.vector.memset(ones1[:], 1.0)

            ACTF = mybir.ActivationFunctionType

            def gather_br(table_ap, b, r):
                rows = int(blk_rows[b, r])
                if rows == 0:
                    return None
                g = gpool.tile([P, rows // P, WTAB], bf16, tag="g", name="g")
                off = int(blk_off[b, r])
                nc.gpsimd.dma_gather(
                    out_ap=g[:],
                    in_ap=table_ap,
                    idxs_ap=idx_sb[:, off // 16:(off + rows) // 16],
                    num_idxs=rows,
                    num_idxs_reg=rows,
                    elem_size=WTAB,
                    single_packet=False,
                    queue_num=qrr(),
                )
                return g

            def phase_gathers(tabA, tabB, blks, rs, store):
                """Issue gathers for given blocks x ranges; store[(b,r)] = tile."""
                for b in blks:
                    for r in rs:
                        src = (tabA[r * RNGW:(r + 1) * RNGW, :] if r < 2 else
                               tabB[(r - 2) * RNGW:(r - 1) * RNGW, :])
                        store[(b, r)] = gather_br(src, b, r)

            def phase1_gathers(blks, rs, store):
                for b in blks:
                    for r in rs:
                        store[(b, r)] = gather_br(t_xt[r * RNGW:(r + 1) * RNGW, :], b, r)

            def cc_half(k, t):
                if t < NTH:
                    return cc_inA[k][t * P:(t + 1) * P, :]
                return cc_inB[k][(t - NTH) * P:(t - NTH + 1) * P, :]

            def phase_compute(ph, gt_store, own_k, w, epilogue, b_lo=0, b_hi=None):
                """Per-block: stream S, own loads, aggregation matmuls, epilogue."""
                for b in range(b_lo, n_blk if b_hi is None else b_hi):
                    u0, u1 = int(blk_use[b, 0]), int(blk_use[b, 1])
                    sS = spool.tile([P, u1 - u0, P], fp8, tag="s", name="sS")
                    nc.scalar.dma_start(out=sS[:], in_=t_S[:, u0:u1, :])
                    for t in range(b * BLK, min((b + 1) * BLK, NT)):
                        own = ep.tile([P, WTAB], bf16, tag="own", name="own")
                        src_ap = (t_xto[t * P:(t + 1) * P, :] if own_k is None
                                  else cc_half(own_k, t))
                        nc.sync.dma_start(out=own[:], in_=src_ap)
                        nmm = 1 + sum(len(uses[t][r]) for r in range(NR))
                        ps = pagg.tile([P, P], f32, tag="pagg", name="ps")
                        nc.tensor.matmul(ps[:w, :], lhsT=own[:, :w], rhs=ident_bf[:],
                                         start=True, stop=(nmm == 1))
                        k = 1
                        for r in range(NR):
                            for (lc, du, _ci) in uses[t][r]:
                                nc.tensor.matmul(
                                    ps[:w, :], lhsT=gt_store[(b, r)][:, lc, :w],
                                    rhs=sS[:, du - u0, :],
                                    start=False, stop=(k == nmm - 1),
                                )
                                k += 1
                        epilogue(t, ps)

            def split_allgather(k):
                nc.gpsimd.collective_compute(
                    "AllGather", mybir.AluOpType.bypass,
                    replica_groups=[list(range(NDEV))],
                    ins=[cc_inA[k][:]], outs=[tabsA[k][:]],
                )

            def split_allgather_b(k):
                nc.gpsimd.collective_compute(
                    "AllGather", mybir.AluOpType.bypass,
                    replica_groups=[list(range(NDEV))],
                    ins=[cc_inB[k][:]], outs=[tabsB[k][:]],
                )

            def dis_mult(ph, t, ps, w):
                a = ep.tile([P, P], bf16, tag="a", name="a")
                nc.vector.tensor_tensor(
                    out=a[:w, :], in0=ps[:w, :],
                    in1=disbc_sb[:w, t * P:(t + 1) * P],
                    op=mybir.AluOpType.mult)
                return a

            # ================= epilogues =================
            def ep1(t, ps):
                a = dis_mult(1, t, ps, D_IN)
                zp = paux.tile([P, P], f32, tag="mm", name="zp")
                nc.tensor.matmul(zp[:], lhsT=a[:D_IN, :], rhs=W1s_sb[:],
                                 start=True, stop=False)
                nc.tensor.matmul(zp[:], lhsT=ones1[:], rhs=t1r_sb[:],
                                 start=False, stop=True)
                hb = ep.tile([P, WTAB], bf16, tag="hb", name="hb")
                nc.scalar.activation(hb[:], zp[:], ACTF.Relu,
                                     scale=dis_sb[:, t:t + 1])
                nc.sync.dma_start(out=cc_half(0, t)[:, :], in_=hb[:])

            def ep2(t, ps):
                a = dis_mult(2, t, ps, 128)
                y3p = paux.tile([P, P], f32, tag="acc", name="y3p", bufs=2)
                hTs = []
                for h in range(2):
                    hp = paux.tile([P, P], f32, tag="mm", name="hp")
                    nc.tensor.matmul(hp[:], lhsT=W2_sb[:, h * 128:(h + 1) * 128],
                                     rhs=a[:], start=True, stop=True)
                    hT = ep.tile([P, P], bf16, tag="hT", name="hT")
                    nc.scalar.activation(hT[:], hp[:], ACTF.Relu,
                                         bias=vecs_sb[:, 2 * h + 1:2 * h + 2],
                                         scale=vecs_sb[:, 2 * h:2 * h + 1])
                    hTs.append(hT)
                for h in range(2):
                    nc.tensor.matmul(y3p[:], lhsT=hTs[h][:],
                                     rhs=W3_sb[:, h * 128:(h + 1) * 128],
                                     start=(h == 0), stop=(h == 1))
                y3b = ep.tile([P, WTAB], bf16, tag="y3b", name="y3b")
                nc.scalar.activation(y3b[:], y3p[:], ACTF.Copy,
                                     scale=dis_sb[:, t:t + 1])
                nc.sync.dma_start(out=cc_half(1, t)[:, :], in_=y3b[:])

            def ep3(t, ps):
                a = dis_mult(3, t, ps, 128)
                h3 = ep.tile([P, P], bf16, tag="h3", name="h3")
                nc.scalar.activation(h3[:], a[:], ACTF.Relu,
                                     bias=vecs_sb[:, 5:6], scale=vecs_sb[:, 4:5])
                y4p = paux.tile([P, 64], f32, tag="mm", name="y4p")
                nc.tensor.matmul(y4p[:], lhsT=h3[:], rhs=W4_sb[:],
                                 start=True, stop=True)
                y4b = ep.tile([P, 64], bf16, tag="y4b", name="y4b")
                nc.scalar.activation(y4b[:], y4p[:], ACTF.Copy,
                                     scale=dis_sb[:, t:t + 1])
                nc.sync.dma_start(out=cc_half(2, t)[:, :64], in_=y4b[:])

            def ep4(t, ps):
                a = dis_mult(4, t, ps, 64)
                h4 = ep.tile([64, P], bf16, tag="h4", name="h4")
                nc.scalar.activation(h4[:], a[:64, :], ACTF.Relu,
                                     bias=vecs_sb[:64, 7:8], scale=vecs_sb[:64, 6:7])
                u1p = paux.tile([64, P], f32, tag="mm", name="u1p")
                nc.tensor.matmul(u1p[:], lhsT=cW1_sb[:], rhs=h4[:], start=True, stop=True)
                u1 = ep.tile([64, P], bf16, tag="u1", name="u1")
                nc.scalar.activation(u1[:], u1p[:], ACTF.Relu, bias=vecs_sb[:64, 8:9])
                u2p = paux.tile([32, P], f32, tag="mm", name="u2p")
                nc.tensor.matmul(u2p[:], lhsT=cW2_sb[:], rhs=u1[:], start=True, stop=True)
                u2 = ep.tile([32, P], bf16, tag="u2", name="u2")
                nc.scalar.activation(u2[:], u2p[:], ACTF.Relu, bias=vecs_sb[:32, 9:10])
                op_ = paux.tile([2, P], f32, tag="mm", name="op_")
                nc.tensor.matmul(op_[:], lhsT=cW3_sb[:], rhs=u2[:], start=True, stop=True)
                oT = ep.tile([2, P], f32, tag="oT", name="oT")
                nc.scalar.activation(oT[:], op_[:], ACTF.Identity, bias=vecs_sb[:2, 10:11])
                nc.sync.dma_start(out=t_out[:, t * P:(t + 1) * P], in_=oT[:])

            # ================= schedule =================
            all_b = list(range(n_blk))
            mid = 9                      # gathers issued before the AG-A trigger
            midc = 7                     # blocks covering tiles 0-48 (half A)
            pre_b = all_b[:PREF_BLK]

            # Program-order contract: every collective is issued AFTER the
            # compute that writes its input (hazard tracking is program-order)
            # but EARLY in the gpsimd stream so it fires mid-phase.

            # ---- phase 1
            g1s = {}
            phase1_gathers(all_b[:mid], range(NR), g1s)
            phase_compute(1, g1s, None, D_IN, ep1, 0, midc)
            split_allgather(0)
            phase1_gathers(all_b[mid:], range(NR), g1s)
            phase_compute(1, g1s, None, D_IN, ep1, midc, n_blk)

            def phase_sched2(k_in, gs, tabA, tabB, ph, own_k, w, ep_f, k_out):
                # prefetch half-A-range gathers for the first blocks (gated
                # only on AG-A(k_in)) before the AG-B trigger so they run
                # during the previous phase's compute tail
                phase_gathers(tabA, tabB, pre_b, (0, 1), gs)
                split_allgather_b(k_in)
                phase_gathers(tabA, tabB, pre_b, (2, 3), gs)
                phase_gathers(tabA, tabB, all_b[PREF_BLK:mid], range(NR), gs)
                phase_compute(ph, gs, own_k, w, ep_f, 0, midc)
                if k_out is not None:
                    split_allgather(k_out)
                phase_gathers(tabA, tabB, all_b[mid:], range(NR), gs)
                phase_compute(ph, gs, own_k, w, ep_f, midc, n_blk)

            g2s = {}
            phase_sched2(0, g2s, tabsA[0], tabsB[0], 2, 0, 128, ep2, 1)
            g3s = {}
            phase_sched2(1, g3s, tabsA[1], tabsB[1], 3, 1, 128, ep3, 2)
            g4s = {}
            phase_sched2(2, g4s, tabsA[2], tabsB[2], 4, 2, 64, ep4, None)

    nc.finalize()
    return nc


_CACHE = {}


def kernel(**inputs):
    x = np.asarray(inputs["x"], dtype=np.float32)
    edge_index = np.asarray(inputs["edge_index"])
    N = x.shape[0]
    key = hashlib.sha256(edge_index.tobytes()).hexdigest()[:16] + f"_{N}_{x.shape[1]}"
    if key not in _CACHE:
        meta = _prep(x, edge_index, inputs, N)
        nc = _build(meta)
        _CACHE[key] = (meta, nc)
    else:
        meta, nc = _CACHE[key]
        meta = dict(meta)
        m2 = _prep(x, edge_index, inputs, N)
        meta.update({k: m2[k] for k in (
            "xt", "vecs", "W1s", "t1row", "W2", "W3", "W4", "cW1", "cW2p", "cW3p",
            "dis_t", "dis_bc")})

    in_maps = []
    for d in range(NDEV):
        in_maps.append({
            "xt": meta["xt"],
            "xt_own": np.concatenate([
                meta["xt"][d * meta["HSH"]:(d + 1) * meta["HSH"]],
                meta["xt"][meta["HALF"] + d * meta["HSH"]:meta["HALF"] + (d + 1) * meta["HSH"]],
            ]),
            "idx": meta["idx_w"][d],
            "S": meta["S_w"][d].astype(nfp8),
            "dis": meta["dis_t"][d],
            "disbc": meta["dis_bc"][d],
            "vecs": meta["vecs"],
            "W1s": meta["W1s"], "t1r": meta["t1row"],
            "W2": meta["W2"], "W3": meta["W3"], "W4": meta["W4"],
            "cW1": meta["cW1"], "cW2p": meta["cW2p"], "cW3p": meta["cW3p"],
        })
    res = None
    for _attempt in range(4):
        try:
            res = run_bass_kernel_spmd(nc, in_maps, core_ids=list(range(NDEV)), trace=False)
            break
        except Exception:
            if _attempt == 3:
                raise

    SHARD = meta["SHARD"]
    out = np.empty((N, 2), dtype=np.float32)
    for d in range(NDEV):
        out[d * SHARD:(d + 1) * SHARD] = res.results[d]["outT"][:, :SHARD].T
    return out
